# revision 1
# baseline (speedup 1.0000x reference)
"""Binarized linear layer (BLinear) Trainium2 kernel.

Computes y = sign(x) @ sign(W).T + b for x [8192, 2048] f32, W [2048, 2048] f32,
b [2048] f32, all data-parallel across 8 NeuronCores (1024 tokens per core,
W replicated).

Math notes:
 - sign() values are in {-1, 0, +1}, exactly representable in bf16.
 - TensorE accumulates in fp32 PSUM; sums of +-1 over K=2048 are exact
   integers << 2^24, so the bf16 matmul is bit-exact vs the fp32 reference.

Per-core pipeline:
 1. DMA fp32 tiles of W ([128 o, 2048 i]) and x ([128 t, 2048 i]) to SBUF.
 2. ScalarE activation(Sign) -> bf16 (handles sign(0)=0 exactly).
 3. HWDGE xbar DMA-transpose (2-byte dtype) into contraction-major layouts
    WbT [128 ki, 16 ko, 2048 o] and xbT [128 ki, 16 ko, 1024 t].
 4. 4 o-banks x 8 t-tiles x 16 k-chunk accumulating matmuls
    (lhsT = xbT k-chunk [128,128], rhs = WbT k-chunk [128,512]) into PSUM.
 5. VectorE tensor_add(psum, bias_bcast) evicts PSUM -> SBUF fp32.
 6. DMA out to y.
"""

import numpy as np

N_CORES = 8
TOKENS = 8192
D_IN = 2048
D_OUT = 2048
T_CORE = TOKENS // N_CORES  # 1024 tokens per core

P = 128
KO = D_IN // P     # 16 contraction chunks
T_TILES = T_CORE // P   # 8 token tiles per core
O_TILES = D_OUT // P    # 16 out-feature tiles
NB = 512           # matmul free dim / PSUM bank
O_BANKS = D_OUT // NB   # 4

_CACHE = {}
LAST_RESULT = None


def _build_bass():
    import concourse.bass as bass
    import concourse.mybir as mybir
    import concourse.tile as tile
    from concourse import bacc
    from concourse.bass import ts

    nc = bacc.Bacc(
        "TRN2",
        target_bir_lowering=False,
        debug=False,
        enable_asserts=False,
    )

    x_d = nc.dram_tensor("x", [T_CORE, D_IN], mybir.dt.float32, kind="ExternalInput")
    w_d = nc.dram_tensor("W", [D_OUT, D_IN], mybir.dt.float32, kind="ExternalInput")
    b_d = nc.dram_tensor("b128", [P, D_OUT], mybir.dt.float32, kind="ExternalInput")
    y_d = nc.dram_tensor("y", [T_CORE, D_OUT], mybir.dt.float32, kind="ExternalOutput")

    x_ap = x_d.ap()
    w_ap = w_d.ap()
    b_ap = b_d.ap()
    y_ap = y_d.ap()

    f32 = mybir.dt.float32
    bf16 = mybir.dt.bfloat16

    with tile.TileContext(nc) as tc:
        with (
            tc.tile_pool(name="persist", bufs=1) as persist,
            tc.tile_pool(name="raw", bufs=3) as raw_pool,
            tc.tile_pool(name="bin", bufs=3) as bin_pool,
            tc.tile_pool(name="outp", bufs=4) as out_pool,
            tc.tile_pool(name="psum", bufs=4, space="PSUM") as psum_pool,
        ):
            wbt = persist.tile([P, KO, D_OUT], bf16)     # [ki, ko, o]
            xbt = persist.tile([P, KO, T_CORE], bf16)    # [ki, ko, t]
            bias = persist.tile([P, D_OUT], f32)

            nc.sync.dma_start(bias[:], b_ap[:, :])

            # W prep: binarize + transpose
            for ot in range(O_TILES):
                w_raw = raw_pool.tile([P, D_IN], f32, tag="raw")
                nc.sync.dma_start(w_raw[:], w_ap[ts(ot, P), :])
                w_bin = bin_pool.tile([P, D_IN], bf16, tag="bin")
                nc.scalar.sign(w_bin[:], w_raw[:])
                nc.sync.dma_start_transpose(wbt[:, :, ts(ot, P)], w_bin[:])

            # x prep: binarize + transpose
            for tt in range(T_TILES):
                x_raw = raw_pool.tile([P, D_IN], f32, tag="raw")
                nc.sync.dma_start(x_raw[:], x_ap[ts(tt, P), :])
                x_bin = bin_pool.tile([P, D_IN], bf16, tag="bin")
                nc.scalar.sign(x_bin[:], x_raw[:])
                nc.sync.dma_start_transpose(xbt[:, :, ts(tt, P)], x_bin[:])

            # matmul: o-bank outer so PE can start after 1/4 of W prep
            for ob in range(O_BANKS):
                for tt in range(T_TILES):
                    psum = psum_pool.tile([P, NB], f32, tag="psum")
                    for k in range(KO):
                        nc.tensor.matmul(
                            psum[:],
                            lhsT=xbt[:, k, ts(tt, P)],
                            rhs=wbt[:, k, ts(ob, NB)],
                            start=(k == 0),
                            stop=(k == KO - 1),
                        )
                    o_sb = out_pool.tile([P, NB], f32, tag="osb")
                    nc.vector.tensor_add(o_sb[:], psum[:], bias[:, ts(ob, NB)])
                    nc.sync.dma_start(y_ap[ts(tt, P), ts(ob, NB)], o_sb[:])

    nc.compile()
    return nc


def _get_nc():
    if "nc" not in _CACHE:
        _CACHE["nc"] = _build_bass()
    return _CACHE["nc"]


def kernel(**inputs):
    global LAST_RESULT
    from concourse.bass_utils import run_bass_kernel_spmd

    x = np.ascontiguousarray(np.asarray(inputs["x"], dtype=np.float32))
    W = np.ascontiguousarray(np.asarray(inputs["W"], dtype=np.float32))
    b = np.ascontiguousarray(np.asarray(inputs["b"], dtype=np.float32))

    b128 = np.ascontiguousarray(np.broadcast_to(b[None, :], (P, D_OUT)))

    nc = _get_nc()
    in_maps = [
        {
            "x": np.ascontiguousarray(x[c * T_CORE : (c + 1) * T_CORE]),
            "W": W,
            "b128": b128,
        }
        for c in range(N_CORES)
    ]
    res = run_bass_kernel_spmd(nc, in_maps, core_ids=list(range(N_CORES)))
    LAST_RESULT = res
    return np.concatenate([r["y"] for r in res.results], axis=0)


# revision 6
# speedup vs baseline: 1.4182x; 1.4182x over previous
"""Binarized linear layer (BLinear) Trainium2 kernel.

Computes y = sign(x) @ sign(W).T + b for x [8192, 2048] f32, W [2048, 2048] f32,
b [2048] f32. Data-parallel across 8 NeuronCores (1024 tokens per core, W
replicated).

Math notes:
 - sign() in {-1, 0, +1} is exact in bf16/fp8e4; TensorE accumulates fp32 in
   PSUM; sums of +-1 over K=2048 are exact integers << 2^24 => bit-exact vs
   the fp32 reference.
 - x and W are staged to DRAM as bf16 (host cast). bf16 has fp32's exponent
   range, so the cast preserves sign()/zeroness for all |v| >= 2^-134 — far
   below anything jax.random.normal produces. This halves HBM traffic and
   enables the 2-byte xbar DMA-transpose directly from DRAM.

Per-core pipeline:
 1. HWDGE xbar DMA-transpose loads straight from DRAM into contraction-major
    SBUF layouts: wv [128 ki, 16 ko, 2048 o], xv [128 ki, 16 ko, 1024 t].
 2. ScalarE activation(Sign) binarizes (fp8e4 out for DoubleRow, or in-place
    bf16).
 3. TensorE matmuls accumulate into PSUM: fp8 DoubleRow (K=256/matmul, 256
    matmuls) or bf16 (K=128, 512 matmuls).
 4. VectorE tensor_add(psum, bias_bcast) evicts PSUM -> SBUF fp32.
 5. DMA out to y.
"""

import numpy as np

N_CORES = 8
TOKENS = 8192
D_IN = 2048
D_OUT = 2048
T_CORE = TOKENS // N_CORES  # 1024 tokens per core

P = 128
KO = D_IN // P     # 16 contraction chunks
T_TILES = T_CORE // P   # 8 token tiles per core
O_TILES = D_OUT // P    # 16 out-feature tiles
NB = 512           # matmul free dim / PSUM bank
O_BANKS = D_OUT // NB   # 4

MM_MODE = "fp8dr"  # "fp8dr" | "bf16"

_CACHE = {}
LAST_RESULT = None


def _build_bass(loop_n=1, phase="all", mm_mode=MM_MODE):
    import concourse.mybir as mybir
    import concourse.tile as tile
    from concourse import bacc
    from concourse.bass import ts

    nc = bacc.Bacc(
        "TRN2",
        target_bir_lowering=False,
        debug=False,
        enable_asserts=False,
    )

    f32 = mybir.dt.float32
    bf16 = mybir.dt.bfloat16
    fp8 = mybir.dt.float8e4

    x_d = nc.dram_tensor("x", [T_CORE, D_IN], bf16, kind="ExternalInput")
    w_d = nc.dram_tensor("W", [D_OUT, D_IN], bf16, kind="ExternalInput")
    b_d = nc.dram_tensor("b128", [P, D_OUT], f32, kind="ExternalInput")
    y_d = nc.dram_tensor("y", [T_CORE, D_OUT], f32, kind="ExternalOutput")

    x_ap = x_d.ap()
    w_ap = w_d.ap()
    b_ap = b_d.ap()
    y_ap = y_d.ap()

    with tile.TileContext(nc) as tc:
        with (
            tc.tile_pool(name="persist", bufs=1) as persist,
            tc.tile_pool(name="outp", bufs=4) as out_pool,
            tc.tile_pool(name="psum", bufs=8, space="PSUM") as psum_pool,
        ):
            wv = persist.tile([P, KO, D_OUT], bf16, name="wv")   # [ki, ko, o]
            xv = persist.tile([P, KO, T_CORE], bf16, name="xv")  # [ki, ko, t]
            bias = persist.tile([P, D_OUT], f32, name="bias")
            if mm_mode == "fp8dr":
                wb = persist.tile([P, KO, D_OUT], fp8, name="wb")
                xb = persist.tile([P, KO, T_CORE], fp8, name="xb")
            else:
                wb, xb = wv, xv
            if phase == "mm":
                nc.gpsimd.memset(wb[:], 1.0)
                nc.gpsimd.memset(xb[:], 1.0)
                nc.gpsimd.memset(bias[:], 0.0)

            def body():
                if phase != "mm":
                    nc.sync.dma_start(bias[:], b_ap[:, :])

                    # x: xbar-transpose straight from DRAM, then binarize
                    for tt in range(T_TILES):
                        nc.sync.dma_start_transpose(
                            xv[:, :, ts(tt, P)], x_ap[ts(tt, P), :])
                        nc.scalar.sign(xb[:, :, ts(tt, P)], xv[:, :, ts(tt, P)])

                    # W: same
                    for ot in range(O_TILES):
                        nc.sync.dma_start_transpose(
                            wv[:, :, ts(ot, P)], w_ap[ts(ot, P), :])
                        nc.scalar.sign(wb[:, :, ts(ot, P)], wv[:, :, ts(ot, P)])

                if phase == "prep":
                    return

                # matmul: o-bank outer so PE can start after 1/4 of W prep
                for ob in range(O_BANKS):
                    for tt in range(T_TILES):
                        psum = psum_pool.tile([P, NB], f32, tag="psum", name="psum")
                        if mm_mode == "fp8dr":
                            for kp in range(KO // 2):
                                nc.tensor.matmul(
                                    psum[:],
                                    lhsT=xb[:, 2 * kp : 2 * kp + 2, ts(tt, P)],
                                    rhs=wb[:, 2 * kp : 2 * kp + 2, ts(ob, NB)],
                                    perf_mode=mybir.MatmulPerfMode.DoubleRow,
                                    start=(kp == 0),
                                    stop=(kp == KO // 2 - 1),
                                )
                        else:
                            for k in range(KO):
                                nc.tensor.matmul(
                                    psum[:],
                                    lhsT=xb[:, k, ts(tt, P)],
                                    rhs=wb[:, k, ts(ob, NB)],
                                    start=(k == 0),
                                    stop=(k == KO - 1),
                                )
                        o_sb = out_pool.tile([P, NB], f32, tag="osb", name="o_sb")
                        nc.vector.tensor_add(o_sb[:], psum[:], bias[:, ts(ob, NB)])
                        nc.sync.dma_start(y_ap[ts(tt, P), ts(ob, NB)], o_sb[:])

            if loop_n > 1:
                with tc.For_i(0, loop_n, 1):
                    body()
            else:
                body()

    nc.compile()
    return nc


def _get_nc():
    if "nc" not in _CACHE:
        _CACHE["nc"] = _build_bass()
    return _CACHE["nc"]


def kernel(**inputs):
    global LAST_RESULT
    import ml_dtypes

    from concourse.bass_utils import run_bass_kernel_spmd

    x = np.asarray(inputs["x"], dtype=np.float32)
    W = np.asarray(inputs["W"], dtype=np.float32)
    b = np.ascontiguousarray(np.asarray(inputs["b"], dtype=np.float32))

    # bf16 staging: sign-preserving (bf16 keeps fp32's exponent range)
    x16 = np.ascontiguousarray(x.astype(ml_dtypes.bfloat16))
    W16 = np.ascontiguousarray(W.astype(ml_dtypes.bfloat16))
    b128 = np.ascontiguousarray(np.broadcast_to(b[None, :], (P, D_OUT)))

    nc = _get_nc()
    in_maps = [
        {
            "x": np.ascontiguousarray(x16[c * T_CORE : (c + 1) * T_CORE]),
            "W": W16,
            "b128": b128,
        }
        for c in range(N_CORES)
    ]
    res = run_bass_kernel_spmd(nc, in_maps, core_ids=list(range(N_CORES)))
    LAST_RESULT = res
    return np.concatenate([r["y"] for r in res.results], axis=0)


# revision 8
# speedup vs baseline: 128621.6057x; 90695.0428x over previous
"""Binarized linear layer (BLinear) Trainium2 kernel.

Computes y = sign(x) @ sign(W).T + b for x [8192, 2048] f32, W [2048, 2048] f32,
b [2048] f32. Data-parallel across 8 NeuronCores (1024 tokens per core, W
replicated).

Math notes:
 - sign() in {-1, 0, +1} is exact in bf16/fp8e4; TensorE accumulates fp32 in
   PSUM; sums of +-1 over K=2048 are exact integers << 2^24 => bit-exact vs
   the fp32 reference.
 - x and W are staged to DRAM as bf16 (host cast). bf16 has fp32's exponent
   range, so the cast preserves sign()/zeroness for all |v| >= 2^-134 — far
   below anything jax.random.normal produces. This halves HBM traffic and
   enables the 2-byte xbar DMA-transpose directly from DRAM.

Per-core pipeline:
 1. HWDGE xbar DMA-transpose loads straight from DRAM into contraction-major
    SBUF layouts: wv [128 ki, 16 ko, 2048 o], xv [128 ki, 16 ko, 1024 t].
 2. ScalarE activation(Sign) binarizes (fp8e4 out for DoubleRow, or in-place
    bf16).
 3. TensorE matmuls accumulate into PSUM: fp8 DoubleRow (K=256/matmul, 256
    matmuls) or bf16 (K=128, 512 matmuls).
 4. VectorE tensor_add(psum, bias_bcast) evicts PSUM -> SBUF fp32.
 5. DMA out to y.
"""

import numpy as np

N_CORES = 8
TOKENS = 8192
D_IN = 2048
D_OUT = 2048
T_CORE = TOKENS // N_CORES  # 1024 tokens per core

P = 128
KO = D_IN // P     # 16 contraction chunks
T_TILES = T_CORE // P   # 8 token tiles per core
O_TILES = D_OUT // P    # 16 out-feature tiles
NB = 512           # matmul free dim / PSUM bank
O_BANKS = D_OUT // NB   # 4

MM_MODE = "fp8dr"  # "fp8dr" | "bf16"

_CACHE = {}
LAST_RESULT = None


def _build_bass(loop_n=1, phase="all", mm_mode=MM_MODE):
    import concourse.mybir as mybir
    import concourse.tile as tile
    from concourse import bacc
    from concourse.bass import ts

    nc = bacc.Bacc(
        "TRN2",
        target_bir_lowering=False,
        debug=False,
        enable_asserts=False,
    )

    f32 = mybir.dt.float32
    bf16 = mybir.dt.bfloat16
    fp8 = mybir.dt.float8e4

    x_d = nc.dram_tensor("x", [T_CORE, D_IN], bf16, kind="ExternalInput")
    w_d = nc.dram_tensor("W", [D_OUT, D_IN], bf16, kind="ExternalInput")
    b_d = nc.dram_tensor("b128", [P, D_OUT], f32, kind="ExternalInput")
    y_d = nc.dram_tensor("y", [T_CORE, D_OUT], f32, kind="ExternalOutput")

    x_ap = x_d.ap()
    w_ap = w_d.ap()
    b_ap = b_d.ap()
    y_ap = y_d.ap()

    with tile.TileContext(nc) as tc:
        with (
            tc.tile_pool(name="persist", bufs=1) as persist,
            tc.tile_pool(name="outp", bufs=4) as out_pool,
            tc.tile_pool(name="psum", bufs=8, space="PSUM") as psum_pool,
        ):
            wv = persist.tile([P, KO, D_OUT], bf16, name="wv")   # [ki, ko, o]
            xv = persist.tile([P, KO, T_CORE], bf16, name="xv")  # [ki, ko, t]
            bias = persist.tile([P, D_OUT], f32, name="bias")
            if mm_mode == "fp8dr":
                wb = persist.tile([P, KO, D_OUT], fp8, name="wb")
                xb = persist.tile([P, KO, T_CORE], fp8, name="xb")
            else:
                wb, xb = wv, xv
            if phase == "mm":
                nc.gpsimd.memset(wb[:], 1.0)
                nc.gpsimd.memset(xb[:], 1.0)
                nc.gpsimd.memset(bias[:], 0.0)

            def body():
                if phase != "mm":
                    # xbar-transpose straight from DRAM in 512-row chunks,
                    # binarize each chunk on ScalarE right behind its load.
                    # Order: x half 0, W bank 0, x half 1, W banks 1-3 so the
                    # first matmul group (ob=0, tt=0..3) unblocks earliest.
                    def prep_x(h):
                        nc.sync.dma_start_transpose(
                            xv[:, :, ts(h, NB)], x_ap[ts(h, NB), :])
                        nc.scalar.sign(xb[:, :, ts(h, NB)], xv[:, :, ts(h, NB)])

                    def prep_w(ob):
                        nc.sync.dma_start_transpose(
                            wv[:, :, ts(ob, NB)], w_ap[ts(ob, NB), :])
                        nc.scalar.sign(wb[:, :, ts(ob, NB)], wv[:, :, ts(ob, NB)])

                    prep_x(0)
                    prep_w(0)
                    prep_x(1)
                    prep_w(1)
                    prep_w(2)
                    prep_w(3)
                    nc.sync.dma_start(bias[:], b_ap[:, :])

                if phase == "prep":
                    return

                # matmul: o-bank outer so PE can start after 1/4 of W prep
                for ob in range(O_BANKS):
                    for tt in range(T_TILES):
                        psum = psum_pool.tile([P, NB], f32, tag="psum", name="psum")
                        if mm_mode == "fp8dr":
                            for kp in range(KO // 2):
                                nc.tensor.matmul(
                                    psum[:],
                                    lhsT=xb[:, 2 * kp : 2 * kp + 2, ts(tt, P)],
                                    rhs=wb[:, 2 * kp : 2 * kp + 2, ts(ob, NB)],
                                    perf_mode=mybir.MatmulPerfMode.DoubleRow,
                                    start=(kp == 0),
                                    stop=(kp == KO // 2 - 1),
                                )
                        else:
                            for k in range(KO):
                                nc.tensor.matmul(
                                    psum[:],
                                    lhsT=xb[:, k, ts(tt, P)],
                                    rhs=wb[:, k, ts(ob, NB)],
                                    start=(k == 0),
                                    stop=(k == KO - 1),
                                )
                        o_sb = out_pool.tile([P, NB], f32, tag="osb", name="o_sb")
                        nc.vector.tensor_add(o_sb[:], psum[:], bias[:, ts(ob, NB)])
                        nc.sync.dma_start(y_ap[ts(tt, P), ts(ob, NB)], o_sb[:])

            if loop_n > 1:
                with tc.For_i(0, loop_n, 1, hint_engines=(mybir.EngineType.PE,)):
                    body()
            else:
                body()

    nc.compile()
    return nc


def _get_nc():
    if "nc" not in _CACHE:
        _CACHE["nc"] = _build_bass()
    return _CACHE["nc"]


def kernel(**inputs):
    global LAST_RESULT
    import ml_dtypes

    from concourse.bass_utils import run_bass_kernel_spmd

    x = np.asarray(inputs["x"], dtype=np.float32)
    W = np.asarray(inputs["W"], dtype=np.float32)
    b = np.ascontiguousarray(np.asarray(inputs["b"], dtype=np.float32))

    # bf16 staging: sign-preserving (bf16 keeps fp32's exponent range)
    x16 = np.ascontiguousarray(x.astype(ml_dtypes.bfloat16))
    W16 = np.ascontiguousarray(W.astype(ml_dtypes.bfloat16))
    b128 = np.ascontiguousarray(np.broadcast_to(b[None, :], (P, D_OUT)))

    nc = _get_nc()
    in_maps = [
        {
            "x": np.ascontiguousarray(x16[c * T_CORE : (c + 1) * T_CORE]),
            "W": W16,
            "b128": b128,
        }
        for c in range(N_CORES)
    ]
    res = run_bass_kernel_spmd(nc, in_maps, core_ids=list(range(N_CORES)))
    LAST_RESULT = res
    return np.concatenate([r["y"] for r in res.results], axis=0)
